# revision 1
# baseline (speedup 1.0000x reference)
"""EnhancedDynamicChannelAttention Trainium2 kernel.

Reference computation (B=16, S=2048, C=1024, H=8, HD=128):
    q[b,h,:]   = pref[b,h]*Wq[:,0] + bq
    k          = f @ Wk.T + bk ;  v = f @ Wv.T + bv       (per head slice)
    scores     = softmax_s(q . k)                          [B,H,S]
    ctx[b,h,:] = sum_s scores * v[b,s,h,:]                 [B,H,HD]
    out        = f + broadcast_s(ctx)

Algebraic folding used here (exact up to fp reassociation):
  - softmax is shift invariant  -> the q.bk term drops entirely.
  - scores[b,h,s] = f[b,s,h,:] . qk[b,h,:]  with  qk = (pref*Wq+bq) @ Wk
  - sum_s attn = 1  ->  ctx = Wv @ (sum_s attn*f[b,s,h,:]) + bv
  So k/v are never materialized; the kernel is memory bound
  (read f once + write out once = 32 MiB per core).

Distribution: pure data parallel over batch, 2 batches per core, 8 cores.

Per-core device program (per batch b, f kept resident in SBUF):
  - DMA in f as 4 x 2MiB super tiles into one [128, 16, 1024] tile
    (s = st*512 + p*4 + t: partition p holds 4 contiguous rows per st).
  - DVE  : tmp = f * qk_bcast ; segmented reduce -> scores [128, 4, 8]
  - ACT  : E = exp(scores)  (no max-sub needed; |scores| < ~30)
  - PE   : uwf[8,1024] += E_t.T @ f_t ; sumE[8,1] += E_t.T @ ones
           (fp32, PSUM accumulation over the 16 sub tiles)
  - tail : uwf /= sumE (row scale) ; per-head PE transpose -> wfT ;
           ctx_row[1,1024] = wfT_h.T @ WvT + bv ; broadcast via DRAM.
  - adds : f += ctx_bcast split between DVE (out of place, half-tile
           grain) and GPSIMD (in place), then DMA out on the ACT ring
           (loads keep the SP ring so the two streams never queue
           behind each other).
"""

import numpy as np

B, S, C = 16, 2048, 1024
H, HD = 8, 128
N_CORES = 8
BPC = B // N_CORES          # batches per core
ST = 4                      # s-rows per partition in a super tile
P = 128
SUP = S // (P * ST)         # super tiles per batch (4)
NT = S // P                 # sub tiles per batch (16)

_CACHE = {}


def _build_program():
    import concourse.bass as bass
    import concourse.bacc as bacc
    import concourse.tile as tile
    from concourse import mybir

    f32 = mybir.dt.float32
    f32r = mybir.dt.float32r

    nc = bacc.Bacc("TRN2", debug=False, num_devices=N_CORES)
    f_in = nc.dram_tensor("features", [BPC, S, C], f32, kind="ExternalInput")
    qk_in = nc.dram_tensor("qkflat", [BPC, C], f32, kind="ExternalInput")
    wvt_in = nc.dram_tensor("wvt", [HD, HD], f32, kind="ExternalInput")
    bvf_in = nc.dram_tensor("bvflat", [1, C], f32, kind="ExternalInput")
    id8_in = nc.dram_tensor("ident8", [8, 8], f32, kind="ExternalInput")
    ones_in = nc.dram_tensor("ones128", [P, 1], f32, kind="ExternalInput")
    out_t = nc.dram_tensor("out", [BPC, S, C], f32, kind="ExternalOutput")

    with tile.TileContext(nc) as tc:
        with (
            tc.tile_pool(name="fpool", bufs=BPC) as fpool,
            tc.tile_pool(name="tmppool", bufs=2) as tmppool,
            tc.tile_pool(name="spool", bufs=2 * SUP) as spool,
            tc.tile_pool(name="small", bufs=2) as small,
            tc.tile_pool(name="singles", bufs=1) as singles,
            tc.tile_pool(name="ps1", bufs=1, space="PSUM") as ps1,
            tc.tile_pool(name="ps2", bufs=2, space="PSUM") as ps2,
            tc.tile_pool(name="dscratch", bufs=2, space="DRAM") as dscratch,
        ):
            wvt_sb = singles.tile([HD, HD], f32)
            nc.scalar.dma_start(out=wvt_sb, in_=wvt_in[:, :])
            bvf_sb = singles.tile([1, C], f32)
            nc.scalar.dma_start(out=bvf_sb, in_=bvf_in[:, :])
            id8_sb = singles.tile([8, 8], f32)
            nc.scalar.dma_start(out=id8_sb, in_=id8_in[:, :])
            ones_sb = singles.tile([P, 1], f32)
            nc.scalar.dma_start(out=ones_sb, in_=ones_in[:, :])

            # qk rows for both batches, broadcast down all partitions.
            # Loaded up front on the SP ring so batch-1 compute is never
            # queued behind batch-0 stores on the ACT ring.
            qk_bcs = []
            for b in range(BPC):
                qk_bc = small.tile([P, C], f32, tag="qkbc")
                nc.sync.dma_start(
                    out=qk_bc, in_=qk_in[b : b + 1, :].to_broadcast([P, C])
                )
                qk_bcs.append(qk_bc)

            for b in range(BPC):
                qk_bc3 = qk_bcs[b].rearrange(
                    "p (o c) -> p o c", o=1
                ).broadcast_to([P, ST, C])

                uwfA = ps2.tile([P, 512], f32, tag="uwfA")
                uwfB = ps2.tile([P, 512], f32, tag="uwfB")
                sumE = ps2.tile([8, 1], f32, tag="sumE", bufs=1)

                fview = f_in[b].rearrange("(st p t) c -> st p t c", p=P, t=ST)
                oview = out_t[b].rearrange("(st p t) c -> st p t c", p=P, t=ST)

                fb = fpool.tile([P, NT, C], f32, tag="fb")
                fb32 = fb
                for st in range(SUP):
                    for half in range(2):
                        lo = st * ST + half * (ST // 2)
                        nc.sync.dma_start(
                            out=fb[:, lo : lo + ST // 2, :],
                            in_=fview[st][:, half * (ST // 2) : (half + 1) * (ST // 2), :],
                        )

                    tmp = tmppool.tile([P, ST, C], f32, tag="tmp")
                    nc.vector.tensor_mul(
                        tmp, fb32[:, st * ST : (st + 1) * ST, :], qk_bc3
                    )
                    scores = spool.tile([P, ST, H], f32, tag="scores")
                    nc.vector.reduce_sum(
                        scores,
                        tmp.rearrange("p t (h d) -> p t h d", h=H),
                        axis=mybir.AxisListType.X,
                    )
                    E_sup = spool.tile([P, ST, H], f32, tag="esup")
                    nc.scalar.activation(
                        out=E_sup.rearrange("p t h -> p (t h)"),
                        in_=scores.rearrange("p t h -> p (t h)"),
                        func=mybir.ActivationFunctionType.Exp,
                    )

                    for t in range(ST):
                        first = st == 0 and t == 0
                        last = st == SUP - 1 and t == ST - 1
                        e_sl = E_sup[:, t, :]
                        f_sl = fb[:, st * ST + t, :]
                        nc.tensor.matmul(
                            uwfA[0:8, :], e_sl, f_sl[:, 0:512],
                            start=first, stop=last,
                        )
                        nc.tensor.matmul(
                            uwfB[0:8, :], e_sl, f_sl[:, 512:1024],
                            start=first, stop=last,
                        )
                        nc.tensor.matmul(
                            sumE, e_sl, ones_sb, start=first, stop=last
                        )

                # ---- tail: ctx_row = (diag(uwf)/sumE) @ WvT + bv ----
                recip = small.tile([8, 1], f32, tag="recip")
                nc.vector.reciprocal(recip, sumE)
                # uwf -> SBUF, normalized rows: uwf[h,:] / sumE[h]
                uwf_sb = small.tile([8, C], f32, tag="uwfsb", bufs=1)
                nc.scalar.copy(out=uwf_sb[:, 0:512], in_=uwfA[0:8, :])
                nc.scalar.copy(out=uwf_sb[:, 512:1024], in_=uwfB[0:8, :])
                nc.vector.tensor_scalar_mul(uwf_sb, uwf_sb, recip)
                # per-head PE transpose into [128, 8*8]; diagonal columns
                # (stride 9) hold wfT[d, h] = uwf[h, h*128+d] / sumE[h]
                wfT8_ps = ps1.tile([P, H * H], f32, tag="wft8")
                for h in range(H):
                    nc.tensor.transpose(
                        wfT8_ps[:, h * H : (h + 1) * H],
                        uwf_sb[:, h * HD : (h + 1) * HD],
                        id8_sb,
                    )
                wfT8_sb = small.tile([P, H * H], f32, tag="wft8sb", bufs=1)
                nc.scalar.copy(out=wfT8_sb, in_=wfT8_ps)

                ctx_ps = ps1.tile([1, C], f32, tag="ctxrow")
                for h in range(H):
                    nc.tensor.matmul(
                        ctx_ps[0:1, h * HD : (h + 1) * HD],
                        wfT8_sb[:, h * (H + 1) : h * (H + 1) + 1],
                        wvt_sb,
                        start=True,
                        stop=True,
                    )
                ctx_row = small.tile([1, C], f32, tag="ctxrowsb", bufs=1)
                nc.vector.tensor_add(ctx_row, ctx_ps, bvf_sb)
                ctx_dram = dscratch.tile([1, C], f32, tag="ctxdram")
                nc.sync.dma_start(out=ctx_dram, in_=ctx_row)
                ctx_bc = small.tile([P, C], f32, tag="ctxbc", bufs=1)
                nc.sync.dma_start(
                    out=ctx_bc, in_=ctx_dram[0:1, :].to_broadcast([P, C])
                )
                ctx_bc3 = ctx_bc.rearrange("p (o c) -> p o c", o=1).broadcast_to(
                    [P, ST, C]
                )

                # residual adds in place, split GPSIMD/DVE per super tile;
                # stores go out on the ACT HWDGE ring so next-batch loads on
                # the SP ring are not queued behind them
                ctx_bc2 = ctx_bc.rearrange("p (o c) -> p o c", o=1).broadcast_to(
                    [P, ST // 2, C]
                )
                for st in range(SUP):
                    fsl32 = fb32[:, st * ST : (st + 1) * ST, :]
                    if st % 2 == 0:
                        # last batch: DVE adds out of place (in-place DVE
                        # adds run 2-5x slower), half-super-tile grain
                        for half in range(2):
                            lo = st * ST + half * (ST // 2)
                            osl = tmppool.tile(
                                [P, ST // 2, C], f32, tag="ostage", bufs=1
                            )
                            nc.vector.tensor_add(
                                osl, fb32[:, lo : lo + ST // 2, :], ctx_bc2
                            )
                            tsl = slice(half * (ST // 2), (half + 1) * (ST // 2))
                            nc.scalar.dma_start(
                                out=oview[st][:, tsl, :], in_=osl
                            )
                    else:
                        nc.gpsimd.tensor_add(fsl32, fsl32, ctx_bc3)
                        nc.scalar.dma_start(out=oview[st], in_=fsl32)

    nc.finalize()
    return nc


def _get_program():
    if "nc" not in _CACHE:
        _CACHE["nc"] = _build_program()
    return _CACHE["nc"]


def _prep_in_maps(features, preference, Wq, bq, Wk, Wv, bv):
    f32 = np.float32
    # qk[b,h,:] = (pref[b,h]*Wq[:,0] + bq) @ Wk   -> flat [B, C]
    q = preference[:, :, None] * Wq[:, 0][None, None, :] + bq  # [B,H,HD]
    qk = np.einsum("bhe,ed->bhd", q, Wk)  # [B,H,HD]
    qkflat = np.ascontiguousarray(qk.reshape(B, C), dtype=f32)
    wvt = np.ascontiguousarray(Wv.T, dtype=f32)
    bvflat = np.ascontiguousarray(np.tile(bv, H)[None, :], dtype=f32)
    id8 = np.eye(8, dtype=f32)
    ones128 = np.ones([P, 1], dtype=f32)

    in_maps = []
    for i in range(N_CORES):
        sl = slice(i * BPC, (i + 1) * BPC)
        in_maps.append(
            {
                "features": np.ascontiguousarray(features[sl], dtype=f32),
                "qkflat": qkflat[sl],
                "wvt": wvt,
                "bvflat": bvflat,
                "ident8": id8,
                "ones128": ones128,
            }
        )
    return in_maps


def kernel(features, preference, Wq, bq, Wk, bk, Wv, bv, **_ignored):
    features = np.asarray(features, dtype=np.float32)
    preference = np.asarray(preference, dtype=np.float32)
    Wq = np.asarray(Wq, dtype=np.float32)
    bq = np.asarray(bq, dtype=np.float32)
    Wk = np.asarray(Wk, dtype=np.float32)
    Wv = np.asarray(Wv, dtype=np.float32)
    bv = np.asarray(bv, dtype=np.float32)

    from concourse.bass_utils import run_bass_kernel_spmd

    nc = _get_program()
    in_maps = _prep_in_maps(features, preference, Wq, bq, Wk, Wv, bv)
    res = run_bass_kernel_spmd(nc, in_maps, core_ids=list(range(N_CORES)))
    out = np.concatenate([r["out"] for r in res.results], axis=0)
    return out.astype(np.float32)



# revision 4
# speedup vs baseline: 1.0657x; 1.0657x over previous
"""EnhancedDynamicChannelAttention Trainium2 kernel.

Reference computation (B=16, S=2048, C=1024, H=8, HD=128):
    q[b,h,:]   = pref[b,h]*Wq[:,0] + bq
    k          = f @ Wk.T + bk ;  v = f @ Wv.T + bv       (per head slice)
    scores     = softmax_s(q . k)                          [B,H,S]
    ctx[b,h,:] = sum_s scores * v[b,s,h,:]                 [B,H,HD]
    out        = f + broadcast_s(ctx)

Algebraic folding (exact up to fp reassociation):
  - softmax is shift invariant  -> the q.bk term drops entirely.
  - scores[b,h,s] = f[b,s,h,:] . qk[b,h,:]  with  qk = (pref*Wq+bq) @ Wk
  - sum_s attn = 1  ->  ctx = Wv @ (sum_s attn*f[b,s,h,:]) + bv
  So k/v are never materialized; the kernel is memory bound.

Distribution: pure data parallel over batch, 2 batches per core, 8 cores.

Per-core device program (per batch b, f kept resident in SBUF):
  - DMA in f as 8 x 1MiB half-super-tiles into one [128, 16, 1024] tile
    (s = st*512 + p*4 + t: partition p holds 4 contiguous rows per st).
  - DVE/GPSIMD: tmp = f * qk_bcast (2 of 8 halves on GPSIMD to keep DVE
    under the DMA floor); DVE segmented reduce -> scores [128, 2, 8]
  - ACT  : E = exp(scores)  (no max-sub needed; |scores| < ~30)
  - PE   : uwf[8,1024] += E_t.T @ f_t ; sumE[8,1] += E_t.T @ ones.
           All matmuls run as float32r (single-pass fp32: 1 col/cycle vs
           4 for LOW_HIGH fp32; products rounded to ~bf16, fp32 PSUM
           accumulate - plenty for the 2e-2 gate).
  - tail : recip = 1/sumE (DVE); ACT copies uwf PSUM->SBUF with
           scale=recip fused; per-head PE transpose -> wfT; ctx_row =
           bv + sum_h wfT_h.T @ WvT accumulated in one PSUM row; ACT
           copies ctx_row to SBUF.  No DRAM roundtrip.
  - residual: out = f + ctx entirely on PE + ACT (DVE has no headroom):
           per 512-col half-subtile, psq = I128.T @ f (identity copy)
           += ones_row.T @ ctx_row (rank-1 broadcast), both f32r;
           ACT copies psq -> bf16 out staging; 2MiB bf16 stores per
           super tile on the ACT HWDGE ring (loads keep the SP ring).
  - output is bf16 (halves store traffic; host upcasts to f32).
"""

import numpy as np

B, S, C = 16, 2048, 1024
H, HD = 8, 128
N_CORES = 8
BPC = B // N_CORES          # batches per core
ST = 4                      # s-rows per partition in a super tile
HST = 2                     # s-rows per half super tile
P = 128
SUP = S // (P * ST)         # super tiles per batch (4)
NT = S // P                 # sub tiles per batch (16)

# which of the 8 half-super-tiles per batch run their score-mul on GPSIMD
GP_HALVES = (1, 4)

_CACHE = {}


def _build_program():
    import concourse.bass as bass
    import concourse.bacc as bacc
    import concourse.tile as tile
    from concourse import mybir

    f32 = mybir.dt.float32
    f32r = mybir.dt.float32r
    bf16 = mybir.dt.bfloat16

    nc = bacc.Bacc("TRN2", debug=False, num_devices=N_CORES)
    f_in = nc.dram_tensor("features", [BPC, S, C], f32r, kind="ExternalInput")
    qk_in = nc.dram_tensor("qkflat", [BPC, C], f32, kind="ExternalInput")
    wvt_in = nc.dram_tensor("wvt", [HD, HD], f32r, kind="ExternalInput")
    bvf_in = nc.dram_tensor("bvflat", [1, C], f32r, kind="ExternalInput")
    id8_in = nc.dram_tensor("ident8", [8, 8], f32, kind="ExternalInput")
    ones_in = nc.dram_tensor("ones128", [P, 2], f32r, kind="ExternalInput")
    id128_in = nc.dram_tensor("ident128", [P, P], f32r, kind="ExternalInput")
    onesr_in = nc.dram_tensor("onesrow", [1, P], f32r, kind="ExternalInput")
    one1_in = nc.dram_tensor("one1", [1, 1], f32r, kind="ExternalInput")
    out_t = nc.dram_tensor("out", [BPC, S, C], bf16, kind="ExternalOutput")

    with tile.TileContext(nc) as tc:
        with (
            tc.tile_pool(name="fpool", bufs=BPC) as fpool,
            tc.tile_pool(name="tmppool", bufs=3) as tmppool,
            tc.tile_pool(name="spool", bufs=4) as spool,
            tc.tile_pool(name="small", bufs=2) as small,
            tc.tile_pool(name="opool", bufs=2) as opool,
            tc.tile_pool(name="singles", bufs=1) as singles,
            tc.tile_pool(name="psU", bufs=1, space="PSUM") as psU,
            tc.tile_pool(name="psR", bufs=2, space="PSUM") as psR,
        ):
            wvt_sb = singles.tile([HD, HD], f32r)
            nc.scalar.dma_start(out=wvt_sb, in_=wvt_in[:, :])
            bvf_sb = singles.tile([1, C], f32r)
            nc.scalar.dma_start(out=bvf_sb, in_=bvf_in[:, :])
            id8_sb = singles.tile([8, 8], f32)
            nc.scalar.dma_start(out=id8_sb, in_=id8_in[:, :])
            ones_sb = singles.tile([P, 2], f32r)
            nc.scalar.dma_start(out=ones_sb, in_=ones_in[:, :])
            id128_sb = singles.tile([P, P], f32r)
            nc.scalar.dma_start(out=id128_sb, in_=id128_in[:, :])
            onesr_sb = singles.tile([1, P], f32r)
            nc.scalar.dma_start(out=onesr_sb, in_=onesr_in[:, :])
            one1_sb = singles.tile([1, 1], f32r)
            nc.scalar.dma_start(out=one1_sb, in_=one1_in[:, :])

            # qk rows for both batches, broadcast down all partitions,
            # loaded up front on the SP ring.
            qk_bcs = []
            for b in range(BPC):
                qk_bc = small.tile([P, C], f32, tag="qkbc")
                nc.sync.dma_start(
                    out=qk_bc, in_=qk_in[b : b + 1, :].to_broadcast([P, C])
                )
                qk_bcs.append(qk_bc)

            for b in range(BPC):
                qk_bc2 = qk_bcs[b].rearrange(
                    "p (o c) -> p o c", o=1
                ).broadcast_to([P, HST, C])

                uwfA = psU.tile([P, 512], f32, tag="uwfA")
                uwfB = psU.tile([P, 512], f32, tag="uwfB")
                sumE = psU.tile([8, 2], f32, tag="sumE")

                fview = f_in[b].rearrange("(st p t) c -> st p t c", p=P, t=ST)
                oview = out_t[b].rearrange("(st p t) c -> st p t c", p=P, t=ST)

                fb = fpool.tile([P, NT, C], f32r, tag="fb")
                for st in range(SUP):
                    for half in range(2):
                        hidx = st * 2 + half
                        lo = st * ST + half * HST
                        nc.sync.dma_start(
                            out=fb[:, lo : lo + HST, :],
                            in_=fview[st][:, half * HST : (half + 1) * HST, :],
                        )
                        tmp = tmppool.tile([P, HST, C], f32, tag="tmp")
                        eng = nc.gpsimd if hidx in GP_HALVES else nc.vector
                        eng.tensor_mul(tmp, fb[:, lo : lo + HST, :].bitcast(f32), qk_bc2)
                        scores = spool.tile([P, HST, H], f32, tag="scores")
                        nc.vector.reduce_sum(
                            scores,
                            tmp.rearrange("p t (h d) -> p t h d", h=H),
                            axis=mybir.AxisListType.X,
                        )
                        E_h = spool.tile([P, HST, H], f32r, tag="esup")
                        nc.scalar.activation(
                            out=E_h.rearrange("p t h -> p (t h)"),
                            in_=scores.rearrange("p t h -> p (t h)"),
                            func=mybir.ActivationFunctionType.Exp,
                        )
                        for t in range(HST):
                            idx = lo + t
                            first = idx == 0
                            last = idx == NT - 1
                            e_r = E_h[:, t, :]
                            f_sl = fb[:, idx, :]
                            nc.tensor.matmul(
                                uwfA[0:8, :], e_r,
                                f_sl[:, 0:512],
                                start=first, stop=last,
                            )
                            nc.tensor.matmul(
                                uwfB[0:8, :], e_r,
                                f_sl[:, 512:1024],
                                start=first, stop=last,
                            )
                            nc.tensor.matmul(
                                sumE, e_r, ones_sb,
                                start=first, stop=last,
                            )

                # ---- tail: ctx_row = bv + (uwf/sumE blockdiag) @ WvT ----
                recip = small.tile([8, 1], f32, tag="recip")
                nc.vector.reciprocal(recip, sumE[0:8, 0:1])
                uwf_sb = small.tile([8, C], f32, tag="uwfsb", bufs=1)
                # PSUM->SBUF copy with the 1/sumE row scale fused on ACT
                nc.scalar.activation(
                    out=uwf_sb[:, 0:512], in_=uwfA[0:8, :],
                    func=mybir.ActivationFunctionType.Copy, scale=recip,
                )
                nc.scalar.activation(
                    out=uwf_sb[:, 512:1024], in_=uwfB[0:8, :],
                    func=mybir.ActivationFunctionType.Copy, scale=recip,
                )
                # per-head PE transpose into [128, 8*8]; diagonal columns
                # (stride 9) hold wfT[d, h] = uwf[h, h*128+d] / sumE[h]
                wfT8_ps = psR.tile([P, 512], f32, tag="resqA")
                for h in range(H):
                    nc.tensor.transpose(
                        wfT8_ps[:, h * H : (h + 1) * H],
                        uwf_sb[:, h * HD : (h + 1) * HD],
                        id8_sb,
                    )
                wfT8_sb = small.tile([P, H * H], f32r, tag="wft8sb", bufs=1)
                nc.scalar.copy(out=wfT8_sb, in_=wfT8_ps[:, 0 : H * H])

                # ctx accumulates into one PSUM row: bv first, then the
                # 8 per-head matvecs
                ctxA_ps = psR.tile([P, 512], f32, tag="resqB")
                ctxB_ps = psR.tile([P, 512], f32, tag="resqA")
                nc.tensor.matmul(
                    ctxA_ps[0:1, :], one1_sb,
                    bvf_sb[0:1, 0:512],
                    start=True, stop=False, skip_group_check=True,
                )
                nc.tensor.matmul(
                    ctxB_ps[0:1, :], one1_sb,
                    bvf_sb[0:1, 512:1024],
                    start=True, stop=False, skip_group_check=True,
                )
                for h in range(H):
                    dst = ctxA_ps if h < 4 else ctxB_ps
                    col = (h % 4) * HD
                    nc.tensor.matmul(
                        dst[0:1, col : col + HD],
                        wfT8_sb[:, h * (H + 1) : h * (H + 1) + 1],
                        wvt_sb,
                        start=False, stop=(h % 4 == 3), skip_group_check=True,
                    )
                ctx_row = small.tile([1, C], f32r, tag="ctxrow_sb", bufs=1)
                nc.scalar.copy(out=ctx_row[0:1, 0:512], in_=ctxA_ps[0:1, :])
                nc.scalar.copy(out=ctx_row[0:1, 512:1024], in_=ctxB_ps[0:1, :])

                # ---- residual out = f + ctx on PE (f32r) + ACT copy ----
                for st in range(SUP):
                    outst = opool.tile([P, ST, C], bf16, tag="outst")
                    for tq in range(ST):
                        idx = st * ST + tq
                        psqA = psR.tile([P, 512], f32, tag="resqA")
                        psqB = psR.tile([P, 512], f32, tag="resqB")
                        nc.tensor.matmul(
                            psqA, id128_sb,
                            fb[:, idx, 0:512],
                            start=True, stop=False, skip_group_check=True,
                        )
                        nc.tensor.matmul(
                            psqA, onesr_sb,
                            ctx_row[0:1, 0:512],
                            start=False, stop=True, skip_group_check=True,
                        )
                        nc.tensor.matmul(
                            psqB, id128_sb,
                            fb[:, idx, 512:1024],
                            start=True, stop=False, skip_group_check=True,
                        )
                        nc.tensor.matmul(
                            psqB, onesr_sb,
                            ctx_row[0:1, 512:1024],
                            start=False, stop=True, skip_group_check=True,
                        )
                        nc.scalar.copy(out=outst[:, tq, 0:512], in_=psqA)
                        nc.scalar.copy(out=outst[:, tq, 512:1024], in_=psqB)
                    nc.scalar.dma_start(out=oview[st], in_=outst)

    nc.finalize()
    return nc


def _get_program():
    if "nc" not in _CACHE:
        _CACHE["nc"] = _build_program()
    return _CACHE["nc"]


def _prep_in_maps(features, preference, Wq, bq, Wk, Wv, bv):
    f32 = np.float32
    # qk[b,h,:] = (pref[b,h]*Wq[:,0] + bq) @ Wk   -> flat [B, C]
    q = preference[:, :, None] * Wq[:, 0][None, None, :] + bq  # [B,H,HD]
    qk = np.einsum("bhe,ed->bhd", q, Wk)  # [B,H,HD]
    qkflat = np.ascontiguousarray(qk.reshape(B, C), dtype=f32)
    wvt = np.ascontiguousarray(Wv.T, dtype=f32)
    bvflat = np.ascontiguousarray(np.tile(bv, H)[None, :], dtype=f32)
    id8 = np.eye(8, dtype=f32)
    ones128 = np.ones([P, 2], dtype=f32)
    id128 = np.eye(P, dtype=f32)
    onesrow = np.ones([1, P], dtype=f32)
    one1 = np.ones([1, 1], dtype=f32)

    in_maps = []
    for i in range(N_CORES):
        sl = slice(i * BPC, (i + 1) * BPC)
        in_maps.append(
            {
                "features": np.ascontiguousarray(features[sl], dtype=f32),
                "qkflat": qkflat[sl],
                "wvt": wvt,
                "bvflat": bvflat,
                "ident8": id8,
                "ones128": ones128,
                "ident128": id128,
                "onesrow": onesrow,
                "one1": one1,
            }
        )
    return in_maps


def kernel(features, preference, Wq, bq, Wk, bk, Wv, bv, **_ignored):
    features = np.asarray(features, dtype=np.float32)
    preference = np.asarray(preference, dtype=np.float32)
    Wq = np.asarray(Wq, dtype=np.float32)
    bq = np.asarray(bq, dtype=np.float32)
    Wk = np.asarray(Wk, dtype=np.float32)
    Wv = np.asarray(Wv, dtype=np.float32)
    bv = np.asarray(bv, dtype=np.float32)

    from concourse.bass_utils import run_bass_kernel_spmd

    nc = _get_program()
    in_maps = _prep_in_maps(features, preference, Wq, bq, Wk, Wv, bv)
    res = run_bass_kernel_spmd(nc, in_maps, core_ids=list(range(N_CORES)))
    out = np.concatenate(
        [np.asarray(r["out"]).astype(np.float32) for r in res.results], axis=0
    )
    return out


# revision 6
# speedup vs baseline: 1.5075x; 1.4146x over previous
"""EnhancedDynamicChannelAttention Trainium2 kernel.

Reference computation (B=16, S=2048, C=1024, H=8, HD=128):
    q[b,h,:]   = pref[b,h]*Wq[:,0] + bq
    k          = f @ Wk.T + bk ;  v = f @ Wv.T + bv       (per head slice)
    scores     = softmax_s(q . k)                          [B,H,S]
    ctx[b,h,:] = sum_s scores * v[b,s,h,:]                 [B,H,HD]
    out        = f + broadcast_s(ctx)

Algebraic folding (exact up to fp reassociation):
  - softmax is shift invariant  -> the q.bk term drops entirely.
  - scores[b,h,s] = f[b,s,h,:] . qk[b,h,:]  with  qk = (pref*Wq+bq) @ Wk
  - sum_s attn = 1  ->  ctx = Wv @ (sum_s attn*f[b,s,h,:]) + bv
  So k/v are never materialized; the kernel is memory bound.

Distribution: pure data parallel over batch, 2 batches per core, 8 cores.

The kernel runs bf16 end to end (measured rel err ~3.5e-3, gate 2e-2):
the PE moving-operand path and the DVE both run 2x faster on 16-bit
data, and bf16 output halves store traffic.  f32 never exists in SBUF:
SWDGE (gpsimd) cast-DMAs convert f32 HBM -> bf16 SBUF inline.

Per-core device program (per batch b):
  - gpsimd cast-DMA loads f as 4 x 2MiB super tiles -> fbh [128,16,1024]
    bf16 (s = st*512 + p*4 + t).
  - DVE  : tmp = fbh * qk_bcast (bf16 2x mode); segmented reduce ->
           scores [128, 2, 8] f32 (fp32 internal accumulate).
  - ACT  : E = exp(scores) -> bf16  (no max-sub needed; |scores| < ~30)
  - PE   : uwf[8,1024] += E_t.T @ f_t ; sumE[8,2] += E_t.T @ ones
           (bf16 operands, 1 col/cycle, fp32 PSUM accumulate).
  - tail : recip = 1/sumE (DVE); ACT copies uwf PSUM->SBUF f32 with
           scale=recip fused; per-head PE transpose -> wfT (bf16 via ACT
           copy); ctx_row = bv + sum_h wfT_h.T @ WvT accumulated in one
           PSUM row; PE rank-1 broadcast ones x ctx_row -> ctxbc
           [128,1024] bf16.  All on-chip, no DRAM roundtrip.
  - residual: outst = fbh + ctxbc_bcast as plain bf16 tensor_tensor
           adds split DVE/GPSIMD; 2MiB bf16 stores per super tile on
           the ACT HWDGE ring.
Emission order keeps every in-order engine queue unblocked: compute b0,
tail b0, compute b1 (cast-DMAs reach the gpsimd queue before b0's
residual TTs), residual b0, tail b1, residual b1.
"""

import numpy as np

B, S, C = 16, 2048, 1024
H, HD = 8, 128
N_CORES = 8
BPC = B // N_CORES          # batches per core
ST = 4                      # s-rows per partition in a super tile
HST = 2                     # s-rows per half super tile
P = 128
SUP = S // (P * ST)         # super tiles per batch (4)
NT = S // P                 # sub tiles per batch (16)

# residual super tiles (b, st) handled by GPSIMD; the rest go to DVE
GP_RES = {(0, 0), (0, 1), (1, 0), (1, 1)}

_CACHE = {}


def _build_program():
    import concourse.bass as bass
    import concourse.bacc as bacc
    import concourse.tile as tile
    from concourse import mybir

    f32 = mybir.dt.float32
    bf16 = mybir.dt.bfloat16
    Exp = mybir.ActivationFunctionType.Exp
    Copy = mybir.ActivationFunctionType.Copy

    nc = bacc.Bacc("TRN2", debug=False, num_devices=N_CORES)
    f_in = nc.dram_tensor("features", [BPC, S, C], f32, kind="ExternalInput")
    qk_in = nc.dram_tensor("qkflat", [BPC, C], bf16, kind="ExternalInput")
    wvt_in = nc.dram_tensor("wvt", [HD, HD], bf16, kind="ExternalInput")
    bvf_in = nc.dram_tensor("bvflat", [1, C], bf16, kind="ExternalInput")
    id8_in = nc.dram_tensor("ident8", [8, 8], f32, kind="ExternalInput")
    ones_in = nc.dram_tensor("ones2", [P, 2], bf16, kind="ExternalInput")
    onesr_in = nc.dram_tensor("onesrow", [1, P], bf16, kind="ExternalInput")
    one1_in = nc.dram_tensor("one1", [1, 1], bf16, kind="ExternalInput")
    out_t = nc.dram_tensor("out", [BPC, S, C], bf16, kind="ExternalOutput")

    with tile.TileContext(nc) as tc:
        with (
            tc.tile_pool(name="fpool", bufs=BPC) as fpool,
            tc.tile_pool(name="tmppool", bufs=3) as tmppool,
            tc.tile_pool(name="spool", bufs=4) as spool,
            tc.tile_pool(name="small", bufs=2) as small,
            tc.tile_pool(name="opool", bufs=4) as opool,
            tc.tile_pool(name="singles", bufs=1) as singles,
            tc.tile_pool(name="psU", bufs=1, space="PSUM") as psU,
            tc.tile_pool(name="psT", bufs=1, space="PSUM") as psT,
        ):
            wvt_sb = singles.tile([HD, HD], bf16)
            nc.scalar.dma_start(out=wvt_sb, in_=wvt_in[:, :])
            bvf_sb = singles.tile([1, C], bf16)
            nc.scalar.dma_start(out=bvf_sb, in_=bvf_in[:, :])
            id8_sb = singles.tile([8, 8], f32)
            nc.scalar.dma_start(out=id8_sb, in_=id8_in[:, :])
            ones_sb = singles.tile([P, 2], bf16)
            nc.scalar.dma_start(out=ones_sb, in_=ones_in[:, :])
            onesr_sb = singles.tile([1, P], bf16)
            nc.scalar.dma_start(out=onesr_sb, in_=onesr_in[:, :])
            one1_sb = singles.tile([1, 1], bf16)
            nc.scalar.dma_start(out=one1_sb, in_=one1_in[:, :])

            qk_bcs = []
            for b in range(BPC):
                qk_bc = small.tile([P, C], bf16, tag="qkbc")
                nc.sync.dma_start(
                    out=qk_bc, in_=qk_in[b : b + 1, :].to_broadcast([P, C])
                )
                qk_bcs.append(qk_bc)

            fbhs = [
                fpool.tile([P, NT, C], bf16, tag="fbh", name=f"fbh{b}")
                for b in range(BPC)
            ]
            uwfA = psU.tile([P, 512], f32, tag="uwfA")
            uwfB = psU.tile([P, 512], f32, tag="uwfB")
            sumE = psU.tile([8, 2], f32, tag="sumE")
            wfT8_ps = psT.tile([P, H * H], f32, tag="wft8")
            ctxA_ps = psT.tile([P, 512], f32, tag="ctxA")
            ctxB_ps = psT.tile([P, 512], f32, tag="ctxB")
            bcA_ps = psT.tile([P, 512], f32, tag="bcA")
            bcB_ps = psT.tile([P, 512], f32, tag="bcB")

            def compute(b):
                """loads + scores + uwf accumulation for batch b"""
                qk_bc2 = qk_bcs[b].rearrange(
                    "p (o c) -> p o c", o=1
                ).broadcast_to([P, HST, C])
                fview = f_in[b].rearrange("(st p t) c -> st p t c", p=P, t=ST)
                fbh = fbhs[b]
                for st in range(SUP):
                    # SWDGE cast-DMA: f32 HBM -> bf16 SBUF, 2 MiB
                    nc.gpsimd.dma_start(
                        out=fbh[:, st * ST : (st + 1) * ST, :], in_=fview[st]
                    )
                    for half in range(2):
                        lo = st * ST + half * HST
                        tmp = tmppool.tile([P, HST, C], bf16, tag="tmp")
                        nc.vector.tensor_mul(
                            tmp, fbh[:, lo : lo + HST, :], qk_bc2
                        )
                        scores = spool.tile([P, HST, H], f32, tag="scores")
                        nc.vector.reduce_sum(
                            scores,
                            tmp.rearrange("p t (h d) -> p t h d", h=H),
                            axis=mybir.AxisListType.X,
                        )
                        E_h = spool.tile([P, HST, H], bf16, tag="esup")
                        nc.scalar.activation(
                            out=E_h.rearrange("p t h -> p (t h)"),
                            in_=scores.rearrange("p t h -> p (t h)"),
                            func=Exp,
                        )
                        for t in range(HST):
                            idx = lo + t
                            first = idx == 0
                            last = idx == NT - 1
                            e_sl = E_h[:, t, :]
                            f_sl = fbh[:, idx, :]
                            nc.tensor.matmul(
                                uwfA[0:8, :], e_sl, f_sl[:, 0:512],
                                start=first, stop=last,
                            )
                            nc.tensor.matmul(
                                uwfB[0:8, :], e_sl, f_sl[:, 512:1024],
                                start=first, stop=last,
                            )
                            nc.tensor.matmul(
                                sumE, e_sl, ones_sb, start=first, stop=last
                            )

            def tail(b):
                """ctx_row + broadcast ctxbc [128, C] bf16 for batch b"""
                recip = small.tile([8, 1], f32, tag="recip")
                nc.vector.reciprocal(recip, sumE[0:8, 0:1])
                uwf_sb = small.tile([8, C], f32, tag="uwfsb", bufs=1)
                nc.scalar.activation(
                    out=uwf_sb[:, 0:512], in_=uwfA[0:8, :], func=Copy,
                    scale=recip,
                )
                nc.scalar.activation(
                    out=uwf_sb[:, 512:1024], in_=uwfB[0:8, :], func=Copy,
                    scale=recip,
                )
                for h in range(H):
                    nc.tensor.transpose(
                        wfT8_ps[:, h * H : (h + 1) * H],
                        uwf_sb[:, h * HD : (h + 1) * HD],
                        id8_sb,
                    )
                wfT8_sb = small.tile([P, H * H], bf16, tag="wft8sb", bufs=1)
                nc.scalar.copy(out=wfT8_sb, in_=wfT8_ps)

                nc.tensor.matmul(
                    ctxA_ps[0:1, :], one1_sb, bvf_sb[0:1, 0:512],
                    start=True, stop=False, skip_group_check=True,
                )
                nc.tensor.matmul(
                    ctxB_ps[0:1, :], one1_sb, bvf_sb[0:1, 512:1024],
                    start=True, stop=False, skip_group_check=True,
                )
                for h in range(H):
                    dst = ctxA_ps if h < 4 else ctxB_ps
                    col = (h % 4) * HD
                    nc.tensor.matmul(
                        dst[0:1, col : col + HD],
                        wfT8_sb[:, h * (H + 1) : h * (H + 1) + 1],
                        wvt_sb,
                        start=False, stop=(h % 4 == 3), skip_group_check=True,
                    )
                ctx_row = small.tile([1, C], bf16, tag="ctxrow", bufs=1)
                nc.scalar.copy(out=ctx_row[0:1, 0:512], in_=ctxA_ps[0:1, :])
                nc.scalar.copy(out=ctx_row[0:1, 512:1024], in_=ctxB_ps[0:1, :])
                # rank-1 PE broadcast down all 128 partitions
                nc.tensor.matmul(
                    bcA_ps, onesr_sb, ctx_row[0:1, 0:512],
                    start=True, stop=True,
                )
                nc.tensor.matmul(
                    bcB_ps, onesr_sb, ctx_row[0:1, 512:1024],
                    start=True, stop=True,
                )
                ctxbc = small.tile([P, C], bf16, tag="ctxbc")
                nc.scalar.copy(out=ctxbc[:, 0:512], in_=bcA_ps)
                nc.scalar.copy(out=ctxbc[:, 512:1024], in_=bcB_ps)
                return ctxbc

            def residual(b, ctxbc):
                """outst = fbh + ctxbc, bf16 stores per super tile"""
                fbh = fbhs[b]
                oview = out_t[b].rearrange("(st p t) c -> st p t c", p=P, t=ST)
                ctx4 = ctxbc.rearrange("p (o c) -> p o c", o=1).broadcast_to(
                    [P, ST, C]
                )
                for st in range(SUP):
                    outst = opool.tile([P, ST, C], bf16, tag="outst")
                    eng = nc.gpsimd if (b, st) in GP_RES else nc.vector
                    eng.tensor_add(
                        outst, fbh[:, st * ST : (st + 1) * ST, :], ctx4
                    )
                    nc.scalar.dma_start(out=oview[st], in_=outst)

            compute(0)
            ctxbc0 = tail(0)
            compute(1)
            residual(0, ctxbc0)
            ctxbc1 = tail(1)
            residual(1, ctxbc1)

    nc.finalize()
    return nc


def _get_program():
    if "nc" not in _CACHE:
        _CACHE["nc"] = _build_program()
    return _CACHE["nc"]


def _prep_in_maps(features, preference, Wq, bq, Wk, Wv, bv):
    import ml_dtypes

    f32 = np.float32
    bf = ml_dtypes.bfloat16
    # qk[b,h,:] = (pref[b,h]*Wq[:,0] + bq) @ Wk   -> flat [B, C]
    q = preference[:, :, None] * Wq[:, 0][None, None, :] + bq  # [B,H,HD]
    qk = np.einsum("bhe,ed->bhd", q, Wk)  # [B,H,HD]
    qkflat = np.ascontiguousarray(qk.reshape(B, C)).astype(bf)
    wvt = np.ascontiguousarray(Wv.T).astype(bf)
    bvflat = np.ascontiguousarray(np.tile(bv, H)[None, :]).astype(bf)
    id8 = np.eye(8, dtype=f32)
    ones2 = np.ones([P, 2], dtype=f32).astype(bf)
    onesrow = np.ones([1, P], dtype=f32).astype(bf)
    one1 = np.ones([1, 1], dtype=f32).astype(bf)

    in_maps = []
    for i in range(N_CORES):
        sl = slice(i * BPC, (i + 1) * BPC)
        in_maps.append(
            {
                "features": np.ascontiguousarray(features[sl], dtype=f32),
                "qkflat": qkflat[sl],
                "wvt": wvt,
                "bvflat": bvflat,
                "ident8": id8,
                "ones2": ones2,
                "onesrow": onesrow,
                "one1": one1,
            }
        )
    return in_maps


def kernel(features, preference, Wq, bq, Wk, bk, Wv, bv, **_ignored):
    features = np.asarray(features, dtype=np.float32)
    preference = np.asarray(preference, dtype=np.float32)
    Wq = np.asarray(Wq, dtype=np.float32)
    bq = np.asarray(bq, dtype=np.float32)
    Wk = np.asarray(Wk, dtype=np.float32)
    Wv = np.asarray(Wv, dtype=np.float32)
    bv = np.asarray(bv, dtype=np.float32)

    from concourse.bass_utils import run_bass_kernel_spmd

    nc = _get_program()
    in_maps = _prep_in_maps(features, preference, Wq, bq, Wk, Wv, bv)
    res = run_bass_kernel_spmd(nc, in_maps, core_ids=list(range(N_CORES)))
    out = np.concatenate(
        [np.asarray(r["out"]).astype(np.float32) for r in res.results], axis=0
    )
    return out
